# revision 31
# baseline (speedup 1.0000x reference)
"""GAT-style multi-head attention (dense adjacency) on 8 TRN2 NeuronCores.

Reference computation:
    h = x @ W.T                       [n, H, d]
    s = h . a_src ; t = h . a_dst     [n, H]
    e[i,j,h] = leaky_relu(s[i,h] + t[j,h], 0.2)
    alpha = softmax_j(where(mask[i,j], e, -inf))
    out[i] = sum_j alpha[i,j,:] h[j]  -> [n, H*d]

Kernel decomposition (per core, core owns a 384-row block of destinations i):
    exp(leaky(z)) for z = s_i + t_j is approximated by a fitted sum of
    exponentials  F(z) ~= sum_k C_k exp(gamma_k z), which is separable:
        p[j,i,h] ~= sum_k  u_k(s_i) * v_k(t_j)
        u_k(s)   = sign(C_k) exp(gamma_k s)
        v_k(t)   = |C_k| exp(gamma_k (t-3))        (-3 cancels in softmax)
    The N x NB x H score tensor is never materialized: the aggregation
    num[i,hd] = sum_j mask[j,i] v_k[j,h] h[j,hd] becomes K matmuls per
    (i-tile, j-tile) with the mask tile as the stationary operand (shared
    by all 8 heads).  k=1 (the dominant exp(z) term) streams in fp16; the
    two correction streams and all softmax denominators stream in fp8
    e4m3 with DoubleRow perf mode (2 j-tiles per pass, 2x PE throughput).

    Engine balance: PE does h = x@W and the mask matmuls; ACT drains h
    PSUM to fp16 SBUF and batch-computes v_k = exp(g_k t + b_k); DVE
    builds the fp16 stream g1 = v1*h in its 2x 16-bit mode; GpSimd builds
    the two fp8 streams; the epilogue combines accumulators with
    u_k(s_i) in fp16 (2x) and divides by the denominator.
"""

import sys

sys.path.insert(0, "/opt/trn_rl_repo")

import math
from contextlib import ExitStack

import numpy as np
import ml_dtypes

import concourse.bacc as bacc
import concourse.bass as bass
import concourse.mybir as mybir
import concourse.tile as tile
from concourse.bass import ts
from concourse.bass_utils import run_bass_kernel_spmd
import concourse.bass_utils as _bu

LDW_OPT = False  # walrus_driver fails with --enable-ldw-opt=true
_orig_run_command = _bu.run_command


def _patched_run_command(argv, **kwargs):
    if LDW_OPT and isinstance(argv, list):
        argv = [
            "--enable-ldw-opt=true" if a == "--enable-ldw-opt=false" else a
            for a in argv
        ]
    return _orig_run_command(argv, **kwargs)


_bu.run_command = _patched_run_command

N, F, H, D = 3072, 512, 8, 64
M = 8  # cores
NB = N // M  # 384 destination rows per core
P = 128
NT = N // P  # 24 row tiles
KT = F // P  # 4 contraction tiles
IT = NB // P  # 3 i-subtiles per core
WC = F + 2 * H  # wcat columns: 512 h (d,h order) | 8 Ws_dst | 8 Ws_src
GRP = 6  # row tiles per t-PSUM group (batched v exponentials)

# fitted approximation exp(leaky_relu(z, 0.2)) ~= sum_k C_k exp(g_k (z-3))
FIT = [(1.0, 1.0), (-0.21543196, 0.4), (0.13797334, 0.2)]

f32 = mybir.dt.float32
f16 = mybir.dt.float16
f8 = mybir.dt.float8e4

TRACE = False
LAST_EXEC_NS = None
LAST_RESULTS = None

_cache = {}

T_MAX = 7.5  # safe bound on |t|, |s| per head
H_MAX = 8.0  # safe bound on |h| elements


def _fp8_pows():
    """Power-of-two prescales keeping fp8 stream values under e4m3 max."""
    pows = []
    for k, (C, g) in enumerate(FIT):
        vmax = abs(C) * math.exp(g * (T_MAX - 3.0))
        bound = vmax if k == 0 else vmax * H_MAX  # k=0 only dden is fp8
        pows.append(int(math.floor(math.log2(224.0 / bound))))
    return pows


def _build_program(debug_dump=False):
    nc = bacc.Bacc("TRN2", target_bir_lowering=False, debug=False, num_devices=M)
    xT_d = nc.dram_tensor("xT", [F, N], f16, kind="ExternalInput").ap()
    xcT_d = nc.dram_tensor("xcT", [F, NB], f16, kind="ExternalInput").ap()
    wcat_d = nc.dram_tensor("wcat", [F, WC], f16, kind="ExternalInput").ap()
    mask8_d = nc.dram_tensor("mask8", [N, NB], f8, kind="ExternalInput").ap()
    out_d = nc.dram_tensor("out", [NB, F], f16, kind="ExternalOutput").ap()

    add = mybir.AluOpType.add
    sub = mybir.AluOpType.subtract
    mult = mybir.AluOpType.mult
    DR = mybir.MatmulPerfMode.DoubleRow
    Exp = mybir.ActivationFunctionType.Exp
    Copy = mybir.ActivationFunctionType.Copy

    assert len(FIT) == 3
    pows = _fp8_pows()
    gammas = [g for _, g in FIT]
    # v_k = exp(g_k t + beta_k) = |C_k| 2^{p_k} exp(g_k (t-3))
    betas = [
        -3.0 * g + math.log(abs(C)) + p * math.log(2.0)
        for (C, g), p in zip(FIT, pows)
    ]
    # u_k = exp(g_k s - (p_k + 6) ln2); 2^-6 guards fp16 overflow in num
    ubiases = [-(p + 6) * math.log(2.0) for p in pows]
    # combine ops by correction sign: num = acc1*u1 (op2) acc2*u2 (op3) ...
    ops = [add if C > 0 else sub for C, _ in FIT]

    with ExitStack() as ctx:
        tc = ctx.enter_context(tile.TileContext(nc))
        const = ctx.enter_context(tc.tile_pool(name="const", bufs=1))
        xpool = ctx.enter_context(tc.tile_pool(name="xpool", bufs=5))
        spool = ctx.enter_context(tc.tile_pool(name="spool", bufs=2))
        acc_pool = ctx.enter_context(tc.tile_pool(name="acc", bufs=1, space="PSUM"))
        stage_ctx = ExitStack()
        ph_pool = stage_ctx.enter_context(tc.tile_pool(name="ph", bufs=2, space="PSUM"))
        pst_pool = stage_ctx.enter_context(
            tc.tile_pool(name="pst", bufs=2, space="PSUM")
        )

        # ---- persistent SBUF ----
        wcat_sb = const.tile([P, KT, WC], f16)
        for kt in range(KT):
            nc.sync.dma_start(
                out=wcat_sb[:, kt, :],
                in_=wcat_d.rearrange("(kt p) c -> p kt c", p=P)[:, kt, :],
            )
        xc_sb = const.tile([P, KT, NB], f16)
        mask16_sb = const.tile([P, NT, NB], f16)
        mask8_sb = const.tile([P, NT, NB], f8)

        h16 = const.tile([P, NT, F], f16)
        g1 = const.tile([P, NT, F], f16)
        g2 = const.tile([P, NT, F], f8)
        g3 = const.tile([P, NT, F], f8)
        dden = const.tile([P, NT, 3, H], f8)
        vsb = const.tile([P, 3, NT, H], f16)
        u16 = const.tile([P, 3, IT, H], f16)
        outf = const.tile([P, IT, F], f16)
        numf = const.tile([P, F], f16)
        den_t = const.tile([P, 3, H], f32)
        rec16 = const.tile([P, H], f16)
        vbias = const.tile([P, 3], f32)
        ubias = const.tile([P, 3], f32)
        for k in range(3):
            nc.vector.memset(vbias[:, k : k + 1], betas[k])
            nc.vector.memset(ubias[:, k : k + 1], ubiases[k])

        # ---- main PSUM accumulators (4 banks each set) ----
        def acc_tiles(pool):
            a1 = pool.tile([P, F], f32, tag="acc1")
            a2 = pool.tile([P, F], f32, tag="acc2")
            a3 = pool.tile([P, F], f32, tag="acc3")
            ad = pool.tile([P, 3 * H], f32, tag="accd")
            return a1, a2, a3, ad

        def acc_step(accs, it, pair):
            a1, a2, a3, ad = accs
            first = pair == 0
            last = pair == NT // 2 - 1

            def k1_part():
                for jj in range(2):
                    jt = 2 * pair + jj
                    nc.tensor.matmul(
                        a1,
                        lhsT=mask16_sb[:, jt, ts(it, P)],
                        rhs=g1[:, jt, :],
                        start=(first and jj == 0),
                        stop=(last and jj == 1),
                        skip_group_check=True,
                    )

            if not last:
                k1_part()
            for a, g in ((a2, g2), (a3, g3)):
                nc.tensor.matmul(
                    a,
                    lhsT=mask8_sb[:, ts(pair, 2), ts(it, P)],
                    rhs=g[:, ts(pair, 2), :],
                    start=first,
                    stop=last,
                    perf_mode=DR,
                    skip_group_check=True,
                )
            nc.tensor.matmul(
                ad,
                lhsT=mask8_sb[:, ts(pair, 2), ts(it, P)],
                rhs=dden[:, ts(pair, 2), :, :],
                start=first,
                stop=last,
                perf_mode=DR,
                skip_group_check=True,
            )
            if last:
                k1_part()

        def epilogue(accs, it, last=False):
            a1, a2, a3, ad = accs
            tmp = spool.tile([P, 2, F], f16, tag="tmp")
            if last:
                srcs = (a1, a2, a3)
            else:
                # fp16 copies of accumulators enable DVE 2x combines
                a16 = spool.tile([P, 3, F], f16, tag="a16")
                for k, a in enumerate((a1, a2, a3)):
                    nc.scalar.activation(out=a16[:, k, :], in_=a, func=Copy)
                srcs = (a16[:, 0, :], a16[:, 1, :], a16[:, 2, :])
            # num = a1*u1 -/+ a2*u2 -/+ a3*u3   (acc cols are (d,h) order;
            # correction accs stop first, so their multiplies go first)
            for k in (1, 2, 0):
                dst = numf if k == 0 else tmp[:, k - 1, :]
                nc.vector.tensor_tensor(
                    out=dst.rearrange("p (d h) -> p d h", h=H),
                    in0=srcs[k].rearrange("p (d h) -> p d h", h=H),
                    in1=u16[:, k, it, :].unsqueeze(1).broadcast_to((P, D, H)),
                    op=mult,
                )
            nc.vector.tensor_tensor(out=numf, in0=numf, in1=tmp[:, 0, :], op=ops[1])
            nc.vector.tensor_tensor(out=numf, in0=numf, in1=tmp[:, 1, :], op=ops[2])
            # den = sum_k (+/-) dden_k * u_k  (on GpSimd, parallel to num;
            # GpSimd cannot read PSUM, so bounce ad through SBUF via ACT)
            ad_sb = spool.tile([P, 3, H], f32, tag="adsb")
            nc.scalar.activation(out=ad_sb, in_=ad, func=Copy)
            for k in range(3):
                nc.gpsimd.tensor_tensor(
                    out=den_t[:, k, :],
                    in0=ad_sb[:, k, :],
                    in1=u16[:, k, it, :],
                    op=mult,
                )
            nc.gpsimd.tensor_tensor(
                out=den_t[:, 0, :], in0=den_t[:, 0, :], in1=den_t[:, 1, :], op=ops[1]
            )
            nc.gpsimd.tensor_tensor(
                out=den_t[:, 0, :], in0=den_t[:, 0, :], in1=den_t[:, 2, :], op=ops[2]
            )
            with nc.allow_low_precision(reason="1/den fits fp16 comfortably"):
                nc.vector.reciprocal(rec16, den_t[:, 0, :])
            # out[i, h*64+d] = num[i, d*8+h] * rec[i, h]
            nc.vector.tensor_tensor(
                out=outf[:, it, :].rearrange("p (h d) -> p h d", h=H),
                in0=numf.rearrange("p (d h) -> p h d", h=H),
                in1=rec16.unsqueeze(2).broadcast_to((P, H, D)),
                op=mult,
            )
            nc.sync.dma_start(out=out_d[ts(it, P), :], in_=outf[:, it, :])

        # ---- stage 0: h tiles; t (and s) into grouped PSUM banks; it=0
        # accumulation trails by one group so the PE never idles
        accs0 = acc_tiles(acc_pool)
        NG = NT // GRP
        S_GRP = 2  # group whose t-PSUM bank carries the s columns
        for grp in range(NG):
            if grp == S_GRP:
                nc.sync.dma_start(
                    out=xc_sb, in_=xcT_d.rearrange("(kt p) i -> p kt i", p=P)
                )
            pt = pst_pool.tile([P, GRP + IT, H], f32, tag="pt")
            for mi in range(GRP):
                mt = grp * GRP + mi
                if mt % 2 == 0:
                    xt_sb = xpool.tile([P, KT, 2 * P], f16, tag="xt")
                    nc.sync.dma_start(
                        out=xt_sb,
                        in_=xT_d[:, ts(mt // 2, 2 * P)].rearrange(
                            "(kt p) m -> p kt m", p=P
                        ),
                    )
                xt_mt = xt_sb[:, :, ts(mt % 2, P)]
                psum_h = ph_pool.tile([P, F], f32, tag="ph")
                for kt in range(KT):
                    nc.tensor.matmul(
                        psum_h,
                        lhsT=xt_mt[:, kt, :],
                        rhs=wcat_sb[:, kt, 0:F],
                        start=(kt == 0),
                        stop=(kt == KT - 1),
                    )
                    nc.tensor.matmul(
                        pt[:, mi, :],
                        lhsT=xt_mt[:, kt, :],
                        rhs=wcat_sb[:, kt, F : F + H],
                        start=(mi == 0 and kt == 0),
                        stop=(mi == GRP - 1 and kt == KT - 1),
                        skip_group_check=True,
                    )
                nc.scalar.activation(out=h16[:, mt, :], in_=psum_h, func=Copy)
                if grp == S_GRP and mi == 0:
                    # s for this core's own i rows, [i, h] layout, rides in
                    # the same (already started) group-0 t bank
                    for it in range(IT):
                        for kt in range(KT):
                            nc.tensor.matmul(
                                pt[:, GRP + it, :],
                                lhsT=xc_sb[:, kt, ts(it, P)],
                                rhs=wcat_sb[:, kt, F + H : F + 2 * H],
                                start=False,
                                stop=(kt == KT - 1),
                                skip_group_check=True,
                            )
            # batched exponentials for the whole group
            nc.sync.dma_start(
                out=mask8_sb[:, ts(grp, GRP), :],
                in_=mask8_d.rearrange("(jt p) i -> p jt i", p=P)[:, ts(grp, GRP), :],
            )
            for k in range(3):
                nc.scalar.activation(
                    out=vsb[:, k, ts(grp, GRP), :],
                    in_=pt[:, 0:GRP, :],
                    func=Exp,
                    scale=gammas[k],
                    bias=vbias[:, k : k + 1],
                )
            if grp == S_GRP:
                for k in range(3):
                    nc.scalar.activation(
                        out=u16[:, k, :, :],
                        in_=pt[:, GRP : GRP + IT, :],
                        func=Exp,
                        scale=gammas[k],
                        bias=ubias[:, k : k + 1],
                    )
            nc.scalar.activation(
                out=dden[:, ts(grp, GRP), :, :],
                in_=vsb[:, :, ts(grp, GRP), :].rearrange("p k m h -> p m k h"),
                func=Copy,
            )
            # weighted streams for the group's row tiles
            for mi in range(GRP):
                mt = grp * GRP + mi
                hv = h16[:, mt, :].rearrange("p (d h) -> p d h", h=H)
                for eng, g_t, k in (
                    (nc.vector, g1, 0),
                    (nc.vector, g2, 1),
                    (nc.vector, g3, 2),
                ):
                    eng.tensor_tensor(
                        out=g_t[:, mt, :].rearrange("p (d h) -> p d h", h=H),
                        in0=hv,
                        in1=vsb[:, k, mt, :].unsqueeze(1).broadcast_to((P, D, H)),
                        op=mult,
                    )
            nc.scalar.activation(
                out=mask16_sb[:, ts(grp, GRP), :],
                in_=mask8_sb[:, ts(grp, GRP), :],
                func=Copy,
            )
            # it0 accumulation trails by one group so the in-order PE
            # never waits on the current group's stream builds
            if grp > 0:
                for pair in range((grp - 1) * GRP // 2, grp * GRP // 2):
                    acc_step(accs0, 0, pair)

        for pair in range((NG - 1) * GRP // 2, NG * GRP // 2):
            acc_step(accs0, 0, pair)

        # ---- remaining i-subtiles + epilogues
        stage_ctx.close()
        accB_pool = ctx.enter_context(tc.tile_pool(name="accB", bufs=1, space="PSUM"))
        epilogue(accs0, 0)
        for it in range(1, IT):
            accs = acc_tiles(accB_pool if it == 1 else acc_pool)
            for pair in range(NT // 2):
                acc_step(accs, it, pair)
            epilogue(accs, it, last=(it == IT - 1))

    nc.compile()
    return nc


def _sim_check(in_map, debug_dump=False):
    """Run the single-core interpreter against one core's inputs (debug aid)."""
    from concourse.bass_interp import CoreSim

    nc = _build_program(debug_dump=debug_dump)
    sim = CoreSim(nc, trace=False)
    for k, v in in_map.items():
        sim.tensor(k)[:] = v
    sim.simulate()
    return {n: np.array(sim.tensor(n)) for n in ["out"]}


def _pack_inputs(x, adj, W, a_src, a_dst):
    x = np.asarray(x, dtype=np.float32)
    adj = np.asarray(adj, dtype=np.int32)
    W = np.asarray(W, dtype=np.float32)
    a_src = np.asarray(a_src, dtype=np.float32)
    a_dst = np.asarray(a_dst, dtype=np.float32)

    xT = np.ascontiguousarray(x.T.astype(np.float16))
    Wr = W.reshape(H, D, F)
    Ws_src = np.einsum("hdf,hd->fh", Wr, a_src).astype(np.float32)
    Ws_dst = np.einsum("hdf,hd->fh", Wr, a_dst).astype(np.float32)
    # h columns in (d, h) order: col d*H+h = W row h*D+d
    Wdh = np.ascontiguousarray(
        W.reshape(H, D, F).transpose(2, 1, 0).reshape(F, D * H)
    )
    wcat = np.concatenate([Wdh, Ws_dst, Ws_src], axis=1).astype(np.float16)
    mask = (adj + np.eye(N, dtype=np.int32)) > 0  # [i, j]
    in_maps = []
    for c in range(M):
        mask_c = np.ascontiguousarray(
            mask[c * NB : (c + 1) * NB, :].T.astype(np.float16)
        )  # [j, i]
        xcT = np.ascontiguousarray(xT[:, c * NB : (c + 1) * NB])
        in_maps.append(
            {
                "xT": xT,
                "xcT": xcT,
                "wcat": wcat,
                "mask8": mask_c.astype(ml_dtypes.float8_e4m3),
            }
        )
    return in_maps


def _install_ntff_hook():
    """Recreate antenv.axon_hooks (absent in this image) so that
    run_bass_kernel_spmd(trace=True) can capture NTFF profiles through
    the axon PJRT .so. Degrades silently when unavailable."""
    import contextlib
    import ctypes
    import os
    import types

    try:
        from antenv.axon_hooks import get_axon_ntff_profile_hook  # noqa: F401

        return True
    except ImportError:
        pass
    so_path = "/opt/axon/libaxon_pjrt.so"
    if not os.path.exists(so_path):
        return False
    lib = ctypes.CDLL(so_path)
    if not hasattr(lib, "axon_start_nrt_profile"):
        return False
    lib.axon_start_nrt_profile.argtypes = [
        ctypes.POINTER(ctypes.c_int64),
        ctypes.c_size_t,
    ]
    lib.axon_start_nrt_profile.restype = ctypes.c_int64
    lib.axon_stop_nrt_profile.argtypes = [ctypes.c_char_p]
    lib.axon_stop_nrt_profile.restype = ctypes.c_int64

    @contextlib.contextmanager
    def _hook(output_dir, device_ids):
        import jax

        jax.devices()
        if device_ids:
            ids = (ctypes.c_int64 * len(device_ids))(*device_ids)
            rc = lib.axon_start_nrt_profile(ids, len(device_ids))
        else:
            rc = lib.axon_start_nrt_profile(None, 0)
        if rc != 0:
            raise RuntimeError(f"axon_start_nrt_profile rc={rc}")
        try:
            yield
        finally:
            n = lib.axon_stop_nrt_profile(str(output_dir).encode())
            print(f"ntff profile: {n} file(s) written to {output_dir}")

    mod = types.ModuleType("antenv.axon_hooks")
    _state = {"hook": _hook}
    mod.get_axon_ntff_profile_hook = lambda: _state["hook"]
    mod.set_axon_ntff_profile_hook = lambda h: _state.__setitem__("hook", h)
    import antenv

    antenv.axon_hooks = mod
    sys.modules["antenv.axon_hooks"] = mod
    return True


def kernel(x, adj, W, a_src, a_dst):
    global LAST_EXEC_NS, LAST_RESULTS
    if "nc" not in _cache:
        _cache["nc"] = _build_program()
    nc = _cache["nc"]
    if TRACE:
        _install_ntff_hook()
    in_maps = _pack_inputs(x, adj, W, a_src, a_dst)
    res = run_bass_kernel_spmd(nc, in_maps, core_ids=list(range(M)), trace=TRACE)
    LAST_EXEC_NS = res.exec_time_ns
    LAST_RESULTS = res
    out = np.concatenate([res.results[c]["out"] for c in range(M)], axis=0)
    return out.astype(np.float32)


# revision 32
# speedup vs baseline: 1.0344x; 1.0344x over previous
"""GAT-style multi-head attention (dense adjacency) on 8 TRN2 NeuronCores.

Reference computation:
    h = x @ W.T                       [n, H, d]
    s = h . a_src ; t = h . a_dst     [n, H]
    e[i,j,h] = leaky_relu(s[i,h] + t[j,h], 0.2)
    alpha = softmax_j(where(mask[i,j], e, -inf))
    out[i] = sum_j alpha[i,j,:] h[j]  -> [n, H*d]

Kernel decomposition (per core, core owns a 384-row block of destinations i):
    exp(leaky(z)) for z = s_i + t_j is approximated by a fitted sum of
    exponentials  F(z) ~= sum_k C_k exp(gamma_k z), which is separable:
        p[j,i,h] ~= sum_k  u_k(s_i) * v_k(t_j)
        u_k(s)   = sign(C_k) exp(gamma_k s)
        v_k(t)   = |C_k| exp(gamma_k (t-3))        (-3 cancels in softmax)
    The N x NB x H score tensor is never materialized: the aggregation
    num[i,hd] = sum_j mask[j,i] v_k[j,h] h[j,hd] becomes K matmuls per
    (i-tile, j-tile) with the mask tile as the stationary operand (shared
    by all 8 heads).  k=1 (the dominant exp(z) term) streams in fp16; the
    two correction streams and all softmax denominators stream in fp8
    e4m3 with DoubleRow perf mode (2 j-tiles per pass, 2x PE throughput).

    Engine balance: PE does h = x@W and the mask matmuls; ACT drains h
    PSUM to fp16 SBUF and batch-computes v_k = exp(g_k t + b_k); DVE
    builds the fp16 stream g1 = v1*h in its 2x 16-bit mode; GpSimd builds
    the two fp8 streams; the epilogue combines accumulators with
    u_k(s_i) in fp16 (2x) and divides by the denominator.
"""

import sys

sys.path.insert(0, "/opt/trn_rl_repo")

import math
from contextlib import ExitStack

import numpy as np
import ml_dtypes

import concourse.bacc as bacc
import concourse.bass as bass
import concourse.mybir as mybir
import concourse.tile as tile
from concourse.bass import ts
from concourse.bass_utils import run_bass_kernel_spmd
import concourse.bass_utils as _bu

LDW_OPT = False  # walrus_driver fails with --enable-ldw-opt=true
_orig_run_command = _bu.run_command


def _patched_run_command(argv, **kwargs):
    if LDW_OPT and isinstance(argv, list):
        argv = [
            "--enable-ldw-opt=true" if a == "--enable-ldw-opt=false" else a
            for a in argv
        ]
    return _orig_run_command(argv, **kwargs)


_bu.run_command = _patched_run_command

N, F, H, D = 3072, 512, 8, 64
M = 8  # cores
NB = N // M  # 384 destination rows per core
P = 128
NT = N // P  # 24 row tiles
KT = F // P  # 4 contraction tiles
IT = NB // P  # 3 i-subtiles per core
WC = F + 2 * H  # wcat columns: 512 h (d,h order) | 8 Ws_dst | 8 Ws_src
GRP = 6  # row tiles per t-PSUM group (batched v exponentials)

# fitted approximation exp(leaky_relu(z, 0.2)) ~= sum_k C_k exp(g_k (z-3))
FIT = [(1.0, 1.0), (-0.21543196, 0.4), (0.13797334, 0.2)]

f32 = mybir.dt.float32
f16 = mybir.dt.float16
f8 = mybir.dt.float8e4

TRACE = False
LAST_EXEC_NS = None
LAST_RESULTS = None

_cache = {}

T_MAX = 7.5  # safe bound on |t|, |s| per head
H_MAX = 8.0  # safe bound on |h| elements


def _fp8_pows():
    """Power-of-two prescales keeping fp8 stream values under e4m3 max."""
    pows = []
    for k, (C, g) in enumerate(FIT):
        vmax = abs(C) * math.exp(g * (T_MAX - 3.0))
        bound = vmax if k == 0 else vmax * H_MAX  # k=0 only dden is fp8
        pows.append(int(math.floor(math.log2(224.0 / bound))))
    return pows


def _build_program(debug_dump=False):
    nc = bacc.Bacc("TRN2", target_bir_lowering=False, debug=False, num_devices=M)
    xT_d = nc.dram_tensor("xT", [F, N], f16, kind="ExternalInput").ap()
    xcT_d = nc.dram_tensor("xcT", [F, NB], f16, kind="ExternalInput").ap()
    wcat_d = nc.dram_tensor("wcat", [F, WC], f16, kind="ExternalInput").ap()
    mask8_d = nc.dram_tensor("mask8", [N, NB], f8, kind="ExternalInput").ap()
    out_d = nc.dram_tensor("out", [NB, F], f16, kind="ExternalOutput").ap()

    add = mybir.AluOpType.add
    sub = mybir.AluOpType.subtract
    mult = mybir.AluOpType.mult
    DR = mybir.MatmulPerfMode.DoubleRow
    Exp = mybir.ActivationFunctionType.Exp
    Copy = mybir.ActivationFunctionType.Copy

    assert len(FIT) == 3
    pows = _fp8_pows()
    gammas = [g for _, g in FIT]
    # v_k = exp(g_k t + beta_k) = |C_k| 2^{p_k} exp(g_k (t-3))
    betas = [
        -3.0 * g + math.log(abs(C)) + p * math.log(2.0)
        for (C, g), p in zip(FIT, pows)
    ]
    # u_k = exp(g_k s - (p_k + 6) ln2); 2^-6 guards fp16 overflow in num
    ubiases = [-(p + 6) * math.log(2.0) for p in pows]
    # combine ops by correction sign: num = acc1*u1 (op2) acc2*u2 (op3) ...
    ops = [add if C > 0 else sub for C, _ in FIT]

    with ExitStack() as ctx:
        tc = ctx.enter_context(tile.TileContext(nc))
        const = ctx.enter_context(tc.tile_pool(name="const", bufs=1))
        xpool = ctx.enter_context(tc.tile_pool(name="xpool", bufs=5))
        spool = ctx.enter_context(tc.tile_pool(name="spool", bufs=2))
        acc_pool = ctx.enter_context(tc.tile_pool(name="acc", bufs=1, space="PSUM"))
        stage_ctx = ExitStack()
        ph_pool = stage_ctx.enter_context(tc.tile_pool(name="ph", bufs=2, space="PSUM"))
        pst_pool = stage_ctx.enter_context(
            tc.tile_pool(name="pst", bufs=2, space="PSUM")
        )

        # ---- persistent SBUF ----
        wcat_sb = const.tile([P, KT, WC], f16)
        nc.sync.dma_start(out=wcat_sb, in_=wcat_d.rearrange("(kt p) c -> p kt c", p=P))
        xc_sb = const.tile([P, KT, NB], f16)
        mask16_sb = const.tile([P, NT, NB], f16)
        mask8_sb = const.tile([P, NT, NB], f8)

        h16 = const.tile([P, NT, F], f16)
        g1 = const.tile([P, NT, F], f16)
        g2 = const.tile([P, NT, F], f8)
        g3 = const.tile([P, NT, F], f8)
        dden = const.tile([P, NT, 3, H], f8)
        vsb = const.tile([P, 3, NT, H], f16)
        u16 = const.tile([P, 3, IT, H], f16)
        outf = const.tile([P, IT, F], f16)
        numf = const.tile([P, F], f16)
        den_t = const.tile([P, 3, H], f32)
        rec16 = const.tile([P, H], f16)
        vbias = const.tile([P, 3], f32)
        ubias = const.tile([P, 3], f32)
        for k in range(3):
            nc.vector.memset(vbias[:, k : k + 1], betas[k])
            nc.vector.memset(ubias[:, k : k + 1], ubiases[k])

        # ---- main PSUM accumulators (4 banks each set) ----
        def acc_tiles(pool):
            a1 = pool.tile([P, F], f32, tag="acc1")
            a2 = pool.tile([P, F], f32, tag="acc2")
            a3 = pool.tile([P, F], f32, tag="acc3")
            ad = pool.tile([P, 3 * H], f32, tag="accd")
            return a1, a2, a3, ad

        def acc_step(accs, it, pair):
            a1, a2, a3, ad = accs
            first = pair == 0
            last = pair == NT // 2 - 1

            def k1_part():
                for jj in range(2):
                    jt = 2 * pair + jj
                    nc.tensor.matmul(
                        a1,
                        lhsT=mask16_sb[:, jt, ts(it, P)],
                        rhs=g1[:, jt, :],
                        start=(first and jj == 0),
                        stop=(last and jj == 1),
                        skip_group_check=True,
                    )

            if not last:
                k1_part()
            for a, g in ((a2, g2), (a3, g3)):
                nc.tensor.matmul(
                    a,
                    lhsT=mask8_sb[:, ts(pair, 2), ts(it, P)],
                    rhs=g[:, ts(pair, 2), :],
                    start=first,
                    stop=last,
                    perf_mode=DR,
                    skip_group_check=True,
                )
            nc.tensor.matmul(
                ad,
                lhsT=mask8_sb[:, ts(pair, 2), ts(it, P)],
                rhs=dden[:, ts(pair, 2), :, :],
                start=first,
                stop=last,
                perf_mode=DR,
                skip_group_check=True,
            )
            if last:
                k1_part()

        def epilogue(accs, it, last=False):
            a1, a2, a3, ad = accs
            tmp = spool.tile([P, 2, F], f16, tag="tmp")
            if last:
                srcs = (a1, a2, a3)
            else:
                # fp16 copies of accumulators enable DVE 2x combines
                a16 = spool.tile([P, 3, F], f16, tag="a16")
                for k, a in enumerate((a1, a2, a3)):
                    nc.scalar.activation(out=a16[:, k, :], in_=a, func=Copy)
                srcs = (a16[:, 0, :], a16[:, 1, :], a16[:, 2, :])
            # num = a1*u1 -/+ a2*u2 -/+ a3*u3   (acc cols are (d,h) order;
            # correction accs stop first, so their multiplies go first)
            for k in (1, 2, 0):
                dst = numf if k == 0 else tmp[:, k - 1, :]
                nc.vector.tensor_tensor(
                    out=dst.rearrange("p (d h) -> p d h", h=H),
                    in0=srcs[k].rearrange("p (d h) -> p d h", h=H),
                    in1=u16[:, k, it, :].unsqueeze(1).broadcast_to((P, D, H)),
                    op=mult,
                )
            nc.vector.tensor_tensor(out=numf, in0=numf, in1=tmp[:, 0, :], op=ops[1])
            nc.vector.tensor_tensor(out=numf, in0=numf, in1=tmp[:, 1, :], op=ops[2])
            # den = sum_k (+/-) dden_k * u_k  (on GpSimd, parallel to num;
            # GpSimd cannot read PSUM, so bounce ad through SBUF via ACT)
            ad_sb = spool.tile([P, 3, H], f32, tag="adsb")
            nc.scalar.activation(out=ad_sb, in_=ad, func=Copy)
            for k in range(3):
                nc.gpsimd.tensor_tensor(
                    out=den_t[:, k, :],
                    in0=ad_sb[:, k, :],
                    in1=u16[:, k, it, :],
                    op=mult,
                )
            nc.gpsimd.tensor_tensor(
                out=den_t[:, 0, :], in0=den_t[:, 0, :], in1=den_t[:, 1, :], op=ops[1]
            )
            nc.gpsimd.tensor_tensor(
                out=den_t[:, 0, :], in0=den_t[:, 0, :], in1=den_t[:, 2, :], op=ops[2]
            )
            with nc.allow_low_precision(reason="1/den fits fp16 comfortably"):
                nc.vector.reciprocal(rec16, den_t[:, 0, :])
            # out[i, h*64+d] = num[i, d*8+h] * rec[i, h]
            nc.vector.tensor_tensor(
                out=outf[:, it, :].rearrange("p (h d) -> p h d", h=H),
                in0=numf.rearrange("p (d h) -> p h d", h=H),
                in1=rec16.unsqueeze(2).broadcast_to((P, H, D)),
                op=mult,
            )
            nc.sync.dma_start(out=out_d[ts(it, P), :], in_=outf[:, it, :])

        # ---- stage 0: h tiles; t (and s) into grouped PSUM banks; it=0
        # accumulation trails by one group so the PE never idles
        accs0 = acc_tiles(acc_pool)
        NG = NT // GRP
        S_GRP = 2  # group whose t-PSUM bank carries the s columns
        for grp in range(NG):
            if grp == S_GRP:
                nc.sync.dma_start(
                    out=xc_sb, in_=xcT_d.rearrange("(kt p) i -> p kt i", p=P)
                )
            nc.sync.dma_start(
                out=mask8_sb[:, ts(grp, GRP), :],
                in_=mask8_d.rearrange("(jt p) i -> p jt i", p=P)[:, ts(grp, GRP), :],
            )
            pt = pst_pool.tile([P, GRP + IT, H], f32, tag="pt")
            for mi in range(GRP):
                mt = grp * GRP + mi
                if mt % 2 == 0:
                    xt_sb = xpool.tile([P, KT, 2 * P], f16, tag="xt")
                    nc.sync.dma_start(
                        out=xt_sb,
                        in_=xT_d[:, ts(mt // 2, 2 * P)].rearrange(
                            "(kt p) m -> p kt m", p=P
                        ),
                    )
                xt_mt = xt_sb[:, :, ts(mt % 2, P)]
                psum_h = ph_pool.tile([P, F], f32, tag="ph")
                for kt in range(KT):
                    nc.tensor.matmul(
                        psum_h,
                        lhsT=xt_mt[:, kt, :],
                        rhs=wcat_sb[:, kt, 0:F],
                        start=(kt == 0),
                        stop=(kt == KT - 1),
                    )
                    nc.tensor.matmul(
                        pt[:, mi, :],
                        lhsT=xt_mt[:, kt, :],
                        rhs=wcat_sb[:, kt, F : F + H],
                        start=(mi == 0 and kt == 0),
                        stop=(mi == GRP - 1 and kt == KT - 1),
                        skip_group_check=True,
                    )
                nc.scalar.activation(out=h16[:, mt, :], in_=psum_h, func=Copy)
                if grp == S_GRP and mi == 0:
                    # s for this core's own i rows, [i, h] layout, rides in
                    # the same (already started) group-0 t bank
                    for it in range(IT):
                        for kt in range(KT):
                            nc.tensor.matmul(
                                pt[:, GRP + it, :],
                                lhsT=xc_sb[:, kt, ts(it, P)],
                                rhs=wcat_sb[:, kt, F + H : F + 2 * H],
                                start=False,
                                stop=(kt == KT - 1),
                                skip_group_check=True,
                            )
            # batched exponentials for the whole group
            for k in range(3):
                nc.scalar.activation(
                    out=vsb[:, k, ts(grp, GRP), :],
                    in_=pt[:, 0:GRP, :],
                    func=Exp,
                    scale=gammas[k],
                    bias=vbias[:, k : k + 1],
                )
            if grp == S_GRP:
                for k in range(3):
                    nc.scalar.activation(
                        out=u16[:, k, :, :],
                        in_=pt[:, GRP : GRP + IT, :],
                        func=Exp,
                        scale=gammas[k],
                        bias=ubias[:, k : k + 1],
                    )
            nc.scalar.activation(
                out=dden[:, ts(grp, GRP), :, :],
                in_=vsb[:, :, ts(grp, GRP), :].rearrange("p k m h -> p m k h"),
                func=Copy,
            )
            # weighted streams for the group's row tiles
            for mi in range(GRP):
                mt = grp * GRP + mi
                hv = h16[:, mt, :].rearrange("p (d h) -> p d h", h=H)
                for eng, g_t, k in (
                    (nc.vector, g1, 0),
                    (nc.vector, g2, 1),
                    (nc.vector, g3, 2),
                ):
                    eng.tensor_tensor(
                        out=g_t[:, mt, :].rearrange("p (d h) -> p d h", h=H),
                        in0=hv,
                        in1=vsb[:, k, mt, :].unsqueeze(1).broadcast_to((P, D, H)),
                        op=mult,
                    )
            nc.scalar.activation(
                out=mask16_sb[:, ts(grp, GRP), :],
                in_=mask8_sb[:, ts(grp, GRP), :],
                func=Copy,
            )
            # it0 accumulation trails by one group so the in-order PE
            # never waits on the current group's stream builds
            if grp > 0:
                for pair in range((grp - 1) * GRP // 2, grp * GRP // 2):
                    acc_step(accs0, 0, pair)

        for pair in range((NG - 1) * GRP // 2, NG * GRP // 2):
            acc_step(accs0, 0, pair)

        # ---- remaining i-subtiles + epilogues
        stage_ctx.close()
        accB_pool = ctx.enter_context(tc.tile_pool(name="accB", bufs=1, space="PSUM"))
        epilogue(accs0, 0)
        for it in range(1, IT):
            accs = acc_tiles(accB_pool if it == 1 else acc_pool)
            for pair in range(NT // 2):
                acc_step(accs, it, pair)
            epilogue(accs, it, last=(it == IT - 1))

    nc.compile()
    return nc


def _sim_check(in_map, debug_dump=False):
    """Run the single-core interpreter against one core's inputs (debug aid)."""
    from concourse.bass_interp import CoreSim

    nc = _build_program(debug_dump=debug_dump)
    sim = CoreSim(nc, trace=False)
    for k, v in in_map.items():
        sim.tensor(k)[:] = v
    sim.simulate()
    return {n: np.array(sim.tensor(n)) for n in ["out"]}


def _pack_inputs(x, adj, W, a_src, a_dst):
    x = np.asarray(x, dtype=np.float32)
    adj = np.asarray(adj, dtype=np.int32)
    W = np.asarray(W, dtype=np.float32)
    a_src = np.asarray(a_src, dtype=np.float32)
    a_dst = np.asarray(a_dst, dtype=np.float32)

    xT = np.ascontiguousarray(x.T.astype(np.float16))
    Wr = W.reshape(H, D, F)
    Ws_src = np.einsum("hdf,hd->fh", Wr, a_src).astype(np.float32)
    Ws_dst = np.einsum("hdf,hd->fh", Wr, a_dst).astype(np.float32)
    # h columns in (d, h) order: col d*H+h = W row h*D+d
    Wdh = np.ascontiguousarray(
        W.reshape(H, D, F).transpose(2, 1, 0).reshape(F, D * H)
    )
    wcat = np.concatenate([Wdh, Ws_dst, Ws_src], axis=1).astype(np.float16)
    mask = (adj + np.eye(N, dtype=np.int32)) > 0  # [i, j]
    in_maps = []
    for c in range(M):
        mask_c = np.ascontiguousarray(
            mask[c * NB : (c + 1) * NB, :].T.astype(np.float16)
        )  # [j, i]
        xcT = np.ascontiguousarray(xT[:, c * NB : (c + 1) * NB])
        in_maps.append(
            {
                "xT": xT,
                "xcT": xcT,
                "wcat": wcat,
                "mask8": mask_c.astype(ml_dtypes.float8_e4m3),
            }
        )
    return in_maps


def _install_ntff_hook():
    """Recreate antenv.axon_hooks (absent in this image) so that
    run_bass_kernel_spmd(trace=True) can capture NTFF profiles through
    the axon PJRT .so. Degrades silently when unavailable."""
    import contextlib
    import ctypes
    import os
    import types

    try:
        from antenv.axon_hooks import get_axon_ntff_profile_hook  # noqa: F401

        return True
    except ImportError:
        pass
    so_path = "/opt/axon/libaxon_pjrt.so"
    if not os.path.exists(so_path):
        return False
    lib = ctypes.CDLL(so_path)
    if not hasattr(lib, "axon_start_nrt_profile"):
        return False
    lib.axon_start_nrt_profile.argtypes = [
        ctypes.POINTER(ctypes.c_int64),
        ctypes.c_size_t,
    ]
    lib.axon_start_nrt_profile.restype = ctypes.c_int64
    lib.axon_stop_nrt_profile.argtypes = [ctypes.c_char_p]
    lib.axon_stop_nrt_profile.restype = ctypes.c_int64

    @contextlib.contextmanager
    def _hook(output_dir, device_ids):
        import jax

        jax.devices()
        if device_ids:
            ids = (ctypes.c_int64 * len(device_ids))(*device_ids)
            rc = lib.axon_start_nrt_profile(ids, len(device_ids))
        else:
            rc = lib.axon_start_nrt_profile(None, 0)
        if rc != 0:
            raise RuntimeError(f"axon_start_nrt_profile rc={rc}")
        try:
            yield
        finally:
            n = lib.axon_stop_nrt_profile(str(output_dir).encode())
            print(f"ntff profile: {n} file(s) written to {output_dir}")

    mod = types.ModuleType("antenv.axon_hooks")
    _state = {"hook": _hook}
    mod.get_axon_ntff_profile_hook = lambda: _state["hook"]
    mod.set_axon_ntff_profile_hook = lambda h: _state.__setitem__("hook", h)
    import antenv

    antenv.axon_hooks = mod
    sys.modules["antenv.axon_hooks"] = mod
    return True


def kernel(x, adj, W, a_src, a_dst):
    global LAST_EXEC_NS, LAST_RESULTS
    if "nc" not in _cache:
        _cache["nc"] = _build_program()
    nc = _cache["nc"]
    if TRACE:
        _install_ntff_hook()
    in_maps = _pack_inputs(x, adj, W, a_src, a_dst)
    res = run_bass_kernel_spmd(nc, in_maps, core_ids=list(range(M)), trace=TRACE)
    LAST_EXEC_NS = res.exec_time_ns
    LAST_RESULTS = res
    out = np.concatenate([res.results[c]["out"] for c in range(M)], axis=0)
    return out.astype(np.float32)
